# revision 34
# baseline (speedup 1.0000x reference)
"""Single-head attention (nn_MultiHeadAttention) Trainium2 Bass kernel.

Full inputs: x [4, 2048, 1024], Wq/Wk/Wv/Wo [1024, 1024], biases [1024].
reference:  q = x @ Wq.T + bq ; k,v likewise
            scores = (q @ k.T) / sqrt(1024) ; attn = softmax(scores, -1)
            out = (attn @ v) @ Wo.T + bo

Weight folding (exact, host-side):
  scores = (x Wq^T + bq)(x Wk^T + bk)^T
         = x (Wq^T Wk) x^T  +  [q.bk per-query const: softmax-invariant]
           + (x Wk^T bq)^T broadcast over queries  + [bq.bk const: invariant]
  so with M = Wq^T Wk, r = (x @ Wk^T bq) * scale:
    scores = (x M) x^T * scale + r[key]          (r folds into the exp bias)
  ctx @ Wo^T + bo = (attn x) (Wv^T Wo^T) + (Wo bv + bo)
  so with P = Wv^T Wo^T, bo' = bo + Wo bv:
    out = (u x) P / Z + bo'
  The kernel computes only 4 matmul phases (768 N=512 matmuls/core instead
  of 1280): A = x_q M ; u = exp(x A^T...) ; G = u^T x ; out = G^T P.

Sharding: 8 cores = 4 batches x 2 query-halves; per-core key order is a
permutation (own half first), to which softmax attention is invariant.

All matmul operands bf16 (PSUM fp32); M/P are folded in fp64 on host then
rounded, measured end-to-end rel err ~4.0e-3 vs the 2e-2 gate.

Startup: DMA transfers round-robin on the wire, so only the first-wave
~1MB (M rows d0-1 + xT d0-1) is issued eagerly; the other M/xT chunks and
all later-phase inputs are released in waves via dep edges on early
A-phase ops. The A phase runs three passes over d (d0-1 copy, d2-3 add,
d4-7 add in separate PSUM groups) so the PE starts as soon as wave 0
lands. A burst of N=128 warmup matmuls holds the PE busy from ~7us so
the HAM clock-gate is at 8/8 before the real stream begins.
"""

import numpy as np
import ml_dtypes
from contextlib import ExitStack

import concourse.bass as bass
import concourse.bacc as bacc
import concourse.mybir as mybir
import concourse.tile as tile
from concourse import bass_utils
from concourse.bass import _add_dep_helper
from concourse.masks import make_identity

F32 = mybir.dt.float32
F32R = mybir.dt.float32r
BF16 = mybir.dt.bfloat16
AF = mybir.ActivationFunctionType
ALU = mybir.AluOpType

B, S, D = 4, 2048, 1024
SQ = S // 2  # queries per core
N_CORES = 8


def build_nc(S=S, D=D, SQ=SQ):
    P = 128
    DT = D // P          # contraction tiles over d (8)
    ET = D // P          # d' tiles (8)
    NBW = min(512, D)    # free-dim block over output features
    NB = D // NBW        # (2)
    SKT = S // P         # key tiles (16)
    SQW = min(512, SQ)
    SQB = SQ // SQW      # (2)
    SQT = SQ // P        # query tiles (8)
    SCALE = 1.0 / float(np.sqrt(D))

    nc = bacc.Bacc("TRN2", target_bir_lowering=False, debug=False)

    xT = nc.dram_tensor("xT", [D, S], BF16, kind="ExternalInput")
    xS = nc.dram_tensor("xS", [S, D], BF16, kind="ExternalInput")
    mT = nc.dram_tensor("mT", [D, D], BF16, kind="ExternalInput")
    pT = nc.dram_tensor("pT", [D, D], BF16, kind="ExternalInput")
    rsc = nc.dram_tensor("rsc", [S], F32, kind="ExternalInput")
    bop = nc.dram_tensor("bop", [D], F32, kind="ExternalInput")
    outd = nc.dram_tensor("out", [SQ, D], F32, kind="ExternalOutput")

    def bcast_ap(handle):
        a = handle[:]
        return bass.AP(tensor=a.tensor, offset=a.offset, ap=[[0, P]] + list(a.ap))

    with tile.TileContext(nc) as tc, ExitStack() as top:
        singles = top.enter_context(tc.tile_pool(name="singles", bufs=1))
        psum_mm = top.enter_context(tc.tile_pool(name="psum_mm", bufs=6, space="PSUM"))
        psum_z = top.enter_context(tc.tile_pool(name="psum_z", bufs=1, space="PSUM"))
        psum_tr = top.enter_context(tc.tile_pool(name="psum_tr", bufs=1, space="PSUM"))

        # Right stack (live to the end; SBUF is plentiful in this pipeline)
        xs_pool = tc.alloc_tile_pool(name="xs", bufs=4, side="right")
        p_pool = tc.alloc_tile_pool(name="p", bufs=2, side="right")
        at_pool = tc.alloc_tile_pool(name="at", bufs=ET, side="right")
        at_tiles = [at_pool.tile([P, SQ], BF16, name=f"at{i}", tag="at")
                    for i in range(ET)]
        gt_pool = tc.alloc_tile_pool(name="gt", bufs=ET, side="right")
        gt_tiles = [gt_pool.tile([P, SQ], BF16, name=f"gt{i}", tag="gt")
                    for i in range(ET)]

        # Left stack
        xt_pool = tc.alloc_tile_pool(name="xt", bufs=1)
        m_pool = tc.alloc_tile_pool(name="m", bufs=2)

        # ------------- input streams -------------
        # Transfers queued together round-robin on the wire, so ONLY the
        # group-A-critical 2MB (m_lo + xta0/1) is issued eagerly; everything
        # else is gated on early A-phase ops via dep edges.
        deferred_dmas = []  # (inst, gate_idx): waits on at_acts[gate_idx]

        # M in three chunks (d0-1, d2-3, d4-7) matching the A-phase passes;
        # wave0 = {m_q0, xta0} = 1MB is the only eager traffic.
        m_chunks = []
        for lo, hi, gate in ((0, 2, None), (2, 4, 0), (4, 8, 4)):
            m = m_pool.tile([P, hi - lo, D], BF16, name=f"m{lo}",
                            tag=f"m{lo}", bufs=1)
            inst = nc.scalar.dma_start(
                out=m,
                in_=mT[lo * P:hi * P, :].rearrange("(t p) e -> p t e", p=P),
            )
            if gate is not None:
                deferred_dmas.append((inst, gate))
            m_chunks.append((lo, m))

        def m_slice(d, et):
            for lo, m in reversed(m_chunks):
                if d >= lo:
                    return m[:, d - lo, et * P:(et + 1) * P]

        xta_tiles = []
        for i in range(DT // 2):
            xta = xt_pool.tile([P, 2, SQ], BF16, name=f"xta{i}", tag="xta",
                               bufs=DT // 2)
            inst = nc.sync.dma_start(
                out=xta,
                in_=xT[i * 2 * P:(i + 1) * 2 * P, 0:SQ].rearrange(
                    "(t p) s -> p t s", p=P),
            )
            if i == 1:
                deferred_dmas.append((inst, 0))
            elif i >= 2:
                deferred_dmas.append((inst, 4))
            xta_tiles.append(xta)

        xtb_tiles = []
        for i in range(2):
            xtb = xt_pool.tile([P, DT // 2, S - SQ], BF16, name=f"xtb{i}",
                               tag="xtb", bufs=2)
            inst = nc.sync.dma_start(
                out=xtb,
                in_=xT[i * (D // 2):(i + 1) * (D // 2), SQ:S].rearrange(
                    "(t p) s -> p t s", p=P),
            )
            deferred_dmas.append((inst, 15))
            xtb_tiles.append(xtb)

        def xt_slice(d, lo, width):
            if lo < SQ:
                return xta_tiles[d // 2][:, d % 2, lo:lo + width]
            return xtb_tiles[d // (DT // 2)][:, d % (DT // 2),
                                            lo - SQ:lo - SQ + width]

        # x in [s, d] layout for the G phase (quad key-tiles)
        xs_quads = []
        for i in range(SKT // 4):
            xs = xs_pool.tile([P, 4, D], BF16, name=f"xs{i}", tag="xs")
            inst = nc.sync.dma_start(
                out=xs,
                in_=xS[i * 4 * P:(i + 1) * 4 * P, :].rearrange(
                    "(t p) d -> p t d", p=P),
            )
            deferred_dmas.append((inst, 15))
            xs_quads.append(xs)

        def xs_slice(sk, dt):
            return xs_quads[sk // 4][:, sk % 4, dt * P:(dt + 1) * P]

        # P column blocks (rhs of the out matmuls); gpsimd queue
        p_cols = []
        for fb in range(NB):
            pc = p_pool.tile([P, DT, NBW], BF16, name="p", tag="p")
            inst = nc.gpsimd.dma_start(
                out=pc,
                in_=pT[:, fb * NBW:(fb + 1) * NBW].rearrange(
                    "(t p) f -> p t f", p=P),
            )
            deferred_dmas.append((inst, 15))
            p_cols.append(pc)

        # small bias layouts
        r_pt = singles.tile([P, SKT], F32, name="r_pt", tag="r_pt")
        nc.gpsimd.dma_start(out=r_pt, in_=rsc[:].rearrange("(t p) -> p t", p=P))
        bo_bc = singles.tile([P, D], F32, name="bo_bc", tag="bo_bc")
        inst = nc.gpsimd.dma_start(out=bo_bc, in_=bcast_ap(bop))
        deferred_dmas.append((inst, 15))

        # constants
        ones_f32 = singles.tile([P, 1], F32, name="ones_f32", tag="ones_f32")
        nc.vector.memset(ones_f32, 1.0)
        ones_col = singles.tile([P, 1], F32R, name="ones_col", tag="ones_col")
        nc.scalar.activation(out=ones_col, in_=ones_f32, func=AF.Copy)
        ident = singles.tile([P, P], F32, name="ident", tag="ident")
        make_identity(nc, ident)
        rzt = singles.tile([P, SQT], F32, name="rzt", tag="rzt")

        # PE warmup: tiny matmuls keep the PE busy while the critical DMA
        # lands, so HAM is at 8/8 when the real stream starts
        # N=128 fp32 matmuls (~210ns cold each) give ~3.4us of sustained PE
        # activity so HAM reaches 8/8 right as the first A-phase data lands.
        # The rhs is a memset tile (ready ~2us before make_identity's ident).
        warm_rhs = singles.tile([P, P], F32, name="warm_rhs", tag="warm_rhs")
        nc.vector.memset(warm_rhs, 0.5)
        wp = psum_tr.tile([1, P], F32, name="wp", tag="tr")
        for _ in range(18):
            nc.tensor.matmul(wp, lhsT=ones_f32, rhs=warm_rhs,
                             start=True, stop=True)

        # ------------- A phase: AT[d', q] = sum_d M[d, d'] xq^T[d, q] ------
        # Three passes over d (d0-1 copy, d2-3 add, d4-7 add) so the PE can
        # start after ~1MB of DMA and never has a matmul gated on a deferred
        # transfer in front of available work.
        at_acts = []
        for d_lo, d_hi in ((0, 2), (2, 4), (4, DT)):
            for et in range(ET):
                for sb in range(SQB):
                    pp = psum_mm.tile([P, SQW], F32, name="pp", tag="mm")
                    for d in range(d_lo, d_hi):
                        nc.tensor.matmul(
                            pp,
                            lhsT=m_slice(d, et),
                            rhs=xt_slice(d, sb * SQW, SQW),
                            start=(d == d_lo), stop=(d == d_hi - 1),
                        )
                    asl = at_tiles[et][:, sb * SQW:(sb + 1) * SQW]
                    if d_lo == 0:
                        cp = nc.scalar.copy(asl, pp)
                        at_acts.append(cp)
                    else:
                        nc.vector.tensor_tensor(
                            out=asl, in0=asl, in1=pp, op=ALU.add)

        # release the deferred DMA issues once the A phase is in flight
        for inst, gate in deferred_dmas:
            _add_dep_helper(inst.ins, at_acts[gate].ins,
                            reason="defer non-critical DMA past startup")

        # ------------- scores: u[sk, q] = exp((x A^T)*scale + r) -----------
        u_pool = tc.alloc_tile_pool(name="u", bufs=SKT * SQB)
        u_tiles = [[None] * SKT for _ in range(SQB)]
        z_pool = tc.alloc_tile_pool(name="ztmp", bufs=2)
        zacc = [z_pool.tile([P, SQW], F32R, name=f"zacc{q}", tag="zacc")
                for q in range(SQB)]
        for sk in range(SKT):
            for q in range(SQB):
                ps = psum_mm.tile([P, SQW], F32, name="ps", tag="mm")
                for e in range(ET):
                    nc.tensor.matmul(
                        ps,
                        lhsT=xt_slice(e, sk * P, P),
                        rhs=at_tiles[e][:, q * SQW:(q + 1) * SQW],
                        start=(e == 0), stop=(e == ET - 1),
                    )
                ut = u_pool.tile([P, SQW], BF16, name=f"u{q}_{sk}", tag="u")
                nc.scalar.activation(out=ut, in_=ps, func=AF.Exp,
                                     bias=r_pt[:, sk:sk + 1], scale=SCALE)
                u_tiles[q][sk] = ut
                if sk == 0:
                    nc.vector.tensor_copy(out=zacc[q], in_=ut)
                else:
                    nc.vector.tensor_tensor(
                        out=zacc[q], in0=zacc[q], in1=ut, op=ALU.add)

        # ------------- G phase: GT[d, q] = sum_sk x[sk, d] u[sk, q] --------
        for dt in range(DT):
            for q in range(SQB):
                pg = psum_mm.tile([P, SQW], F32, name="pg", tag="mm")
                for sk in range(SKT):
                    nc.tensor.matmul(
                        pg,
                        lhsT=xs_slice(sk, dt),
                        rhs=u_tiles[q][sk],
                        start=(sk == 0), stop=(sk == SKT - 1),
                    )
                nc.scalar.copy(gt_tiles[dt][:, q * SQW:(q + 1) * SQW], pg)

        # ------------- Z finalize: partition-sum, transpose, 1/Z ----------
        with tc.tile_pool(name="zfin", bufs=1) as zf_pool:
            for q in range(SQB):
                pz = psum_z.tile([1, SQW], F32, name="pz", tag="z")
                nc.tensor.matmul(pz, lhsT=(ones_col), rhs=(zacc[q]),
                                 start=True, stop=True)
                z_sb = zf_pool.tile([1, SQW], F32, name="z_sb", tag="z_sb")
                nc.scalar.copy(z_sb, pz)
                for j in range(SQW // P):
                    pt = psum_tr.tile([P, 1], F32, name="pt", tag="tr")
                    nc.tensor.transpose(
                        pt, z_sb[0:1, j * P:(j + 1) * P], ident[0:1, 0:1])
                    jj = q * (SQW // P) + j
                    nc.vector.reciprocal(out=rzt[:, jj:jj + 1], in_=pt)

        # ------------- out: out[q, f] = (sum_d GT[d,q] P[d,f]) / Z + bo' ---
        with tc.tile_pool(name="ofly", bufs=3) as o_pool:
            for fb in range(NB):
                for st in range(SQT):
                    po = psum_mm.tile([P, NBW], F32, name="po", tag="mm")
                    for dt in range(DT):
                        nc.tensor.matmul(
                            po,
                            lhsT=gt_tiles[dt][:, st * P:(st + 1) * P],
                            rhs=p_cols[fb][:, dt, :],
                            start=(dt == 0), stop=(dt == DT - 1),
                        )
                    # the very last block drains in two halves so the kernel
                    # tail (last STT + store) is shorter
                    nh = 2 if (fb == NB - 1 and st == SQT - 1) else 1
                    hw = NBW // nh
                    for h in range(nh):
                        osb = o_pool.tile([P, hw], F32, name="osb", tag="ofly")
                        nc.vector.scalar_tensor_tensor(
                            out=osb, in0=po[:, h * hw:(h + 1) * hw],
                            scalar=rzt[:, st:st + 1],
                            in1=bo_bc[:, fb * NBW + h * hw:fb * NBW + (h + 1) * hw],
                            op0=ALU.mult, op1=ALU.add,
                        )
                        nc.scalar.dma_start(
                            out=outd[st * P:(st + 1) * P,
                                     fb * NBW + h * hw:fb * NBW + (h + 1) * hw],
                            in_=osb,
                        )

        # releases (LIFO per side)
        z_pool.release()
        u_pool.release()
        m_pool.release()
        xt_pool.release()
        gt_pool.release()
        at_pool.release()
        p_pool.release()
        xs_pool.release()

    nc.compile()
    return nc


_NC_CACHE = {}


def _get_nc():
    if "nc" not in _NC_CACHE:
        _NC_CACHE["nc"] = build_nc()
    return _NC_CACHE["nc"]


def _bf16(a):
    return np.ascontiguousarray(np.asarray(a, np.float32)).astype(ml_dtypes.bfloat16)


def make_in_maps(x, Wq, bq, Wk, bk, Wv, bv, Wo, bo):
    x = np.asarray(x, dtype=np.float32)
    Wq = np.asarray(Wq, np.float64)
    Wk = np.asarray(Wk, np.float64)
    Wv = np.asarray(Wv, np.float64)
    Wo = np.asarray(Wo, np.float64)
    # exact host-side weight folds
    M = _bf16(Wq.T @ Wk)                       # [d, d']
    Pm = _bf16(Wv.T @ Wo.T)                    # [d, f]
    w2 = (Wk.T @ np.asarray(bq, np.float64))   # [d'] key-bias fold
    bo_p = np.ascontiguousarray(
        (np.asarray(bo, np.float64) + Wo @ np.asarray(bv, np.float64))
        .astype(np.float32))
    scale = 1.0 / np.sqrt(D)

    in_maps = []
    for c in range(N_CORES):
        b, h = c // 2, c % 2
        xb = x[b]  # [S, D]
        mine = xb[h * SQ:(h + 1) * SQ]
        other = xb[(1 - h) * SQ:(2 - h) * SQ]
        xperm = np.concatenate([mine, other], axis=0)  # [S, D] key order
        rscv = np.ascontiguousarray(
            ((xperm.astype(np.float64) @ w2) * scale).astype(np.float32))
        in_maps.append({
            "xT": _bf16(xperm.T), "xS": _bf16(xperm),
            "mT": M, "pT": Pm, "rsc": rscv, "bop": bo_p,
        })
    return in_maps


def assemble(results):
    out = np.empty((B, S, D), np.float32)
    for c in range(N_CORES):
        b, h = c // 2, c % 2
        out[b, h * SQ:(h + 1) * SQ] = results[c]["out"]
    return out


def kernel(x, Wq, bq, Wk, bk, Wv, bv, Wo, bo, **kwargs):
    nc = _get_nc()
    in_maps = make_in_maps(x, Wq, bq, Wk, bk, Wv, bv, Wo, bo)
    res = bass_utils.run_bass_kernel_spmd(nc, in_maps, core_ids=list(range(N_CORES)))
    return assemble(res.results)


# revision 39
# speedup vs baseline: 1.0175x; 1.0175x over previous
"""Single-head attention (nn_MultiHeadAttention) Trainium2 Bass kernel.

Full inputs: x [4, 2048, 1024], Wq/Wk/Wv/Wo [1024, 1024], biases [1024].
reference:  q = x @ Wq.T + bq ; k,v likewise
            scores = (q @ k.T) / sqrt(1024) ; attn = softmax(scores, -1)
            out = (attn @ v) @ Wo.T + bo

Weight folding (exact, host-side):
  scores = (x Wq^T + bq)(x Wk^T + bk)^T
         = x (Wq^T Wk) x^T  +  [q.bk per-query const: softmax-invariant]
           + (x Wk^T bq)^T broadcast over queries  + [bq.bk const: invariant]
  so with M = Wq^T Wk, r = (x @ Wk^T bq) * scale:
    scores = (x M) x^T * scale + r[key]          (r folds into the exp bias)
  ctx @ Wo^T + bo = (attn x) (Wv^T Wo^T) + (Wo bv + bo)
  so with P = Wv^T Wo^T, bo' = bo + Wo bv:
    out = (u x) P / Z + bo'
  The kernel computes only 4 matmul phases (768 N=512 matmuls/core instead
  of 1280): A = x_q M ; u = exp(x A^T...) ; G = u^T x ; out = G^T P.

Sharding: 8 cores = 4 batches x 2 query-halves; per-core key order is a
permutation (own half first), to which softmax attention is invariant.

All matmul operands bf16 (PSUM fp32); M/P are folded in fp64 on host then
rounded, measured end-to-end rel err ~4.0e-3 vs the 2e-2 gate.

Startup: DMA transfers round-robin on the wire, so only the first-wave
~1MB (M rows d0-1 + xT d0-1) is issued eagerly; the other M/xT chunks and
all later-phase inputs are released in waves via dep edges on early
A-phase ops. The A phase runs three passes over d (d0-1 copy, d2-3 add,
d4-7 add in separate PSUM groups) so the PE starts as soon as wave 0
lands. A burst of N=128 warmup matmuls holds the PE busy from ~7us so
the HAM clock-gate is at 8/8 before the real stream begins.
"""

import numpy as np
import ml_dtypes
from contextlib import ExitStack

import concourse.bass as bass
import concourse.bacc as bacc
import concourse.mybir as mybir
import concourse.tile as tile
from concourse import bass_utils
from concourse.bass import _add_dep_helper
from concourse.masks import make_identity

F32 = mybir.dt.float32
F32R = mybir.dt.float32r
BF16 = mybir.dt.bfloat16
AF = mybir.ActivationFunctionType
ALU = mybir.AluOpType

B, S, D = 4, 2048, 1024
SQ = S // 2  # queries per core
N_CORES = 8


def build_nc(S=S, D=D, SQ=SQ):
    P = 128
    DT = D // P          # contraction tiles over d (8)
    ET = D // P          # d' tiles (8)
    NBW = min(512, D)    # free-dim block over output features
    NB = D // NBW        # (2)
    SKT = S // P         # key tiles (16)
    SQW = min(512, SQ)
    SQB = SQ // SQW      # (2)
    SQT = SQ // P        # query tiles (8)
    SCALE = 1.0 / float(np.sqrt(D))

    nc = bacc.Bacc("TRN2", target_bir_lowering=False, debug=False)

    xT = nc.dram_tensor("xT", [D, S], BF16, kind="ExternalInput")
    xS = nc.dram_tensor("xS", [S, D], BF16, kind="ExternalInput")
    mT = nc.dram_tensor("mT", [D, D], BF16, kind="ExternalInput")
    pT = nc.dram_tensor("pT", [D, D], BF16, kind="ExternalInput")
    rsc = nc.dram_tensor("rsc", [S], F32, kind="ExternalInput")
    bop = nc.dram_tensor("bop", [D], F32, kind="ExternalInput")
    outd = nc.dram_tensor("out", [SQ, D], F32, kind="ExternalOutput")

    def bcast_ap(handle):
        a = handle[:]
        return bass.AP(tensor=a.tensor, offset=a.offset, ap=[[0, P]] + list(a.ap))

    with tile.TileContext(nc) as tc, ExitStack() as top:
        singles = top.enter_context(tc.tile_pool(name="singles", bufs=1))
        dram = top.enter_context(tc.tile_pool(name="dram", bufs=1, space="DRAM"))
        psum_mm = top.enter_context(tc.tile_pool(name="psum_mm", bufs=7, space="PSUM"))
        psum_z = top.enter_context(tc.tile_pool(name="psum_z", bufs=1, space="PSUM"))

        # Right stack (live to the end; SBUF is plentiful in this pipeline)
        xs_pool = tc.alloc_tile_pool(name="xs", bufs=4, side="right")
        p_pool = tc.alloc_tile_pool(name="p", bufs=2, side="right")
        at_pool = tc.alloc_tile_pool(name="at", bufs=ET, side="right")
        at_tiles = [at_pool.tile([P, SQ], BF16, name=f"at{i}", tag="at")
                    for i in range(ET)]
        gt_pool = tc.alloc_tile_pool(name="gt", bufs=ET, side="right")
        gt_tiles = [gt_pool.tile([P, SQ], BF16, name=f"gt{i}", tag="gt")
                    for i in range(ET)]

        # Left stack
        xt_pool = tc.alloc_tile_pool(name="xt", bufs=1)
        m_pool = tc.alloc_tile_pool(name="m", bufs=2)

        # ------------- input streams -------------
        # Transfers queued together round-robin on the wire, so ONLY the
        # group-A-critical 2MB (m_lo + xta0/1) is issued eagerly; everything
        # else is gated on early A-phase ops via dep edges.
        deferred_dmas = []  # (inst, gate_idx): waits on at_acts[gate_idx]

        # M in three chunks (d0-1, d2-3, d4-7) matching the A-phase passes;
        # wave0 = {m_q0, xta0} = 1MB is the only eager traffic.
        m_chunks = []
        for lo, hi, gate in ((0, 2, None), (2, 4, 0), (4, 8, 4)):
            m = m_pool.tile([P, hi - lo, D], BF16, name=f"m{lo}",
                            tag=f"m{lo}", bufs=1)
            inst = nc.scalar.dma_start(
                out=m,
                in_=mT[lo * P:hi * P, :].rearrange("(t p) e -> p t e", p=P),
            )
            if gate is not None:
                deferred_dmas.append((inst, gate))
            m_chunks.append((lo, m))

        def m_slice(d, et):
            for lo, m in reversed(m_chunks):
                if d >= lo:
                    return m[:, d - lo, et * P:(et + 1) * P]

        xta_tiles = []
        for i in range(DT // 2):
            xta = xt_pool.tile([P, 2, SQ], BF16, name=f"xta{i}", tag="xta",
                               bufs=DT // 2)
            inst = nc.sync.dma_start(
                out=xta,
                in_=xT[i * 2 * P:(i + 1) * 2 * P, 0:SQ].rearrange(
                    "(t p) s -> p t s", p=P),
            )
            if i == 1:
                deferred_dmas.append((inst, 0))
            elif i >= 2:
                deferred_dmas.append((inst, 4))
            xta_tiles.append(xta)

        xtb_tiles = []
        for i in range(2):
            xtb = xt_pool.tile([P, DT // 2, S - SQ], BF16, name=f"xtb{i}",
                               tag="xtb", bufs=2)
            inst = nc.sync.dma_start(
                out=xtb,
                in_=xT[i * (D // 2):(i + 1) * (D // 2), SQ:S].rearrange(
                    "(t p) s -> p t s", p=P),
            )
            deferred_dmas.append((inst, 15))
            xtb_tiles.append(xtb)

        def xt_slice(d, lo, width):
            if lo < SQ:
                return xta_tiles[d // 2][:, d % 2, lo:lo + width]
            return xtb_tiles[d // (DT // 2)][:, d % (DT // 2),
                                            lo - SQ:lo - SQ + width]

        # x in [s, d] layout for the G phase (quad key-tiles)
        xs_quads = []
        for i in range(SKT // 4):
            xs = xs_pool.tile([P, 4, D], BF16, name=f"xs{i}", tag="xs")
            inst = nc.sync.dma_start(
                out=xs,
                in_=xS[i * 4 * P:(i + 1) * 4 * P, :].rearrange(
                    "(t p) d -> p t d", p=P),
            )
            deferred_dmas.append((inst, 15))
            xs_quads.append(xs)

        def xs_slice(sk, dt):
            return xs_quads[sk // 4][:, sk % 4, dt * P:(dt + 1) * P]

        # P column blocks (rhs of the out matmuls); gpsimd queue
        p_cols = []
        for fb in range(NB):
            pc = p_pool.tile([P, DT, NBW], BF16, name="p", tag="p")
            inst = nc.gpsimd.dma_start(
                out=pc,
                in_=pT[:, fb * NBW:(fb + 1) * NBW].rearrange(
                    "(t p) f -> p t f", p=P),
            )
            deferred_dmas.append((inst, 15))
            p_cols.append(pc)

        # small bias layouts
        r_pt = singles.tile([P, SKT], F32, name="r_pt", tag="r_pt")
        nc.gpsimd.dma_start(out=r_pt, in_=rsc[:].rearrange("(t p) -> p t", p=P))
        bo_bc = singles.tile([P, D], F32, name="bo_bc", tag="bo_bc")
        inst = nc.gpsimd.dma_start(out=bo_bc, in_=bcast_ap(bop))
        deferred_dmas.append((inst, 15))

        # constants
        ones_f32 = singles.tile([P, 1], F32, name="ones_f32", tag="ones_f32")
        nc.vector.memset(ones_f32, 1.0)
        ones_col = singles.tile([P, 1], F32R, name="ones_col", tag="ones_col")
        nc.scalar.activation(out=ones_col, in_=ones_f32, func=AF.Copy)
        rzt = singles.tile([P, SQT], F32, name="rzt", tag="rzt")

        # PE warmup: tiny matmuls keep the PE busy while the critical DMA
        # lands, so HAM is at 8/8 when the real stream starts
        # N=128 fp32 matmuls (~210ns cold each) give ~3.4us of sustained PE
        # activity so HAM reaches 8/8 right as the first A-phase data lands.
        # The rhs is a memset tile (ready ~2us before make_identity's ident).
        warm_rhs = singles.tile([P, P], F32, name="warm_rhs", tag="warm_rhs")
        nc.vector.memset(warm_rhs, 0.5)
        wp = psum_z.tile([1, P], F32, name="wp", tag="z")
        for _ in range(18):
            nc.tensor.matmul(wp, lhsT=ones_f32, rhs=warm_rhs,
                             start=True, stop=True)

        # ------------- A phase: AT[d', q] = sum_d M[d, d'] xq^T[d, q] ------
        # Three passes over d (d0-1 copy, d2-3 add, d4-7 add) so the PE can
        # start after ~1MB of DMA and never has a matmul gated on a deferred
        # transfer in front of available work.
        at_acts = []
        for d_lo, d_hi in ((0, 2), (2, 4), (4, DT)):
            for et in range(ET):
                for sb in range(SQB):
                    pp = psum_mm.tile([P, SQW], F32, name="pp", tag="mm")
                    for d in range(d_lo, d_hi):
                        nc.tensor.matmul(
                            pp,
                            lhsT=m_slice(d, et),
                            rhs=xt_slice(d, sb * SQW, SQW),
                            start=(d == d_lo), stop=(d == d_hi - 1),
                        )
                    asl = at_tiles[et][:, sb * SQW:(sb + 1) * SQW]
                    if d_lo == 0:
                        cp = nc.scalar.copy(asl, pp)
                        at_acts.append(cp)
                    else:
                        nc.vector.tensor_tensor(
                            out=asl, in0=asl, in1=pp, op=ALU.add)

        # release the deferred DMA issues once the A phase is in flight
        for inst, gate in deferred_dmas:
            _add_dep_helper(inst.ins, at_acts[gate].ins,
                            reason="defer non-critical DMA past startup")

        # ------------- scores: u[sk, q] = exp((x A^T)*scale + r) -----------
        u_pool = tc.alloc_tile_pool(name="u", bufs=SKT * SQB)
        u_tiles = [[None] * SKT for _ in range(SQB)]
        z_pool = tc.alloc_tile_pool(name="ztmp", bufs=2)
        zacc = [z_pool.tile([P, SQW], F32R, name=f"zacc{q}", tag="zacc")
                for q in range(SQB)]
        for sk in range(SKT):
            for q in range(SQB):
                ps = psum_mm.tile([P, SQW], F32, name="ps", tag="mm")
                for e in range(ET):
                    nc.tensor.matmul(
                        ps,
                        lhsT=xt_slice(e, sk * P, P),
                        rhs=at_tiles[e][:, q * SQW:(q + 1) * SQW],
                        start=(e == 0), stop=(e == ET - 1),
                    )
                ut = u_pool.tile([P, SQW], BF16, name=f"u{q}_{sk}", tag="u")
                nc.scalar.activation(out=ut, in_=ps, func=AF.Exp,
                                     bias=r_pt[:, sk:sk + 1], scale=SCALE)
                u_tiles[q][sk] = ut
                if sk == 0:
                    nc.vector.tensor_copy(out=zacc[q], in_=ut)
                else:
                    nc.vector.tensor_tensor(
                        out=zacc[q], in0=zacc[q], in1=ut, op=ALU.add)

        # ------------- G phase: GT[d, q] = sum_sk x[sk, d] u[sk, q] --------
        for dt in range(DT):
            for q in range(SQB):
                pg = psum_mm.tile([P, SQW], F32, name="pg", tag="mm")
                for sk in range(SKT):
                    nc.tensor.matmul(
                        pg,
                        lhsT=xs_slice(sk, dt),
                        rhs=u_tiles[q][sk],
                        start=(sk == 0), stop=(sk == SKT - 1),
                    )
                nc.scalar.copy(gt_tiles[dt][:, q * SQW:(q + 1) * SQW], pg)

        # ------------- Z finalize: partition-sum, DMA-transpose, 1/Z ------
        # Z [1, SQ] is laid across query partitions via a DRAM bounce (the
        # same fast (t p)->p t gather as the bias loads), keeping the PE out
        # of the transpose entirely.
        zd = dram.tile([SQ], F32, name="zd", tag="zd")
        z_sb = singles.tile([1, SQ], F32, name="z_sb", tag="z_sb")
        for q in range(SQB):
            pz = psum_z.tile([1, SQW], F32, name="pz", tag="z")
            nc.tensor.matmul(pz, lhsT=(ones_col), rhs=(zacc[q]),
                             start=True, stop=True)
            nc.scalar.copy(z_sb[:, q * SQW:(q + 1) * SQW], pz)
            nc.scalar.dma_start(out=zd[q * SQW:(q + 1) * SQW],
                                in_=z_sb[:, q * SQW:(q + 1) * SQW])
        z_raw = singles.tile([P, SQT], F32, name="z_raw", tag="z_raw")
        nc.gpsimd.dma_start(out=z_raw, in_=zd[:].rearrange("(t p) -> p t", p=P))
        nc.vector.reciprocal(out=rzt, in_=z_raw)

        # ------------- out: out[q, f] = (sum_d GT[d,q] P[d,f]) / Z + bo' ---
        with tc.tile_pool(name="ofly", bufs=3) as o_pool:
            for fb in range(NB):
                for st in range(SQT):
                    po = psum_mm.tile([P, NBW], F32, name="po", tag="mm")
                    for dt in range(DT):
                        nc.tensor.matmul(
                            po,
                            lhsT=gt_tiles[dt][:, st * P:(st + 1) * P],
                            rhs=p_cols[fb][:, dt, :],
                            start=(dt == 0), stop=(dt == DT - 1),
                        )
                    # the very last block drains in two halves so the kernel
                    # tail (last STT + store) is shorter
                    nh = 2 if (fb == NB - 1 and st == SQT - 1) else 1
                    hw = NBW // nh
                    for h in range(nh):
                        osb = o_pool.tile([P, hw], F32, name="osb", tag="ofly")
                        nc.vector.scalar_tensor_tensor(
                            out=osb, in0=po[:, h * hw:(h + 1) * hw],
                            scalar=rzt[:, st:st + 1],
                            in1=bo_bc[:, fb * NBW + h * hw:fb * NBW + (h + 1) * hw],
                            op0=ALU.mult, op1=ALU.add,
                        )
                        nc.scalar.dma_start(
                            out=outd[st * P:(st + 1) * P,
                                     fb * NBW + h * hw:fb * NBW + (h + 1) * hw],
                            in_=osb,
                        )

        # releases (LIFO per side)
        z_pool.release()
        u_pool.release()
        m_pool.release()
        xt_pool.release()
        gt_pool.release()
        at_pool.release()
        p_pool.release()
        xs_pool.release()

    nc.compile()
    return nc


_NC_CACHE = {}


def _get_nc():
    if "nc" not in _NC_CACHE:
        _NC_CACHE["nc"] = build_nc()
    return _NC_CACHE["nc"]


def _bf16(a):
    return np.ascontiguousarray(np.asarray(a, np.float32)).astype(ml_dtypes.bfloat16)


def make_in_maps(x, Wq, bq, Wk, bk, Wv, bv, Wo, bo):
    x = np.asarray(x, dtype=np.float32)
    Wq = np.asarray(Wq, np.float64)
    Wk = np.asarray(Wk, np.float64)
    Wv = np.asarray(Wv, np.float64)
    Wo = np.asarray(Wo, np.float64)
    # exact host-side weight folds
    M = _bf16(Wq.T @ Wk)                       # [d, d']
    Pm = _bf16(Wv.T @ Wo.T)                    # [d, f]
    w2 = (Wk.T @ np.asarray(bq, np.float64))   # [d'] key-bias fold
    bo_p = np.ascontiguousarray(
        (np.asarray(bo, np.float64) + Wo @ np.asarray(bv, np.float64))
        .astype(np.float32))
    scale = 1.0 / np.sqrt(D)

    in_maps = []
    for c in range(N_CORES):
        b, h = c // 2, c % 2
        xb = x[b]  # [S, D]
        mine = xb[h * SQ:(h + 1) * SQ]
        other = xb[(1 - h) * SQ:(2 - h) * SQ]
        xperm = np.concatenate([mine, other], axis=0)  # [S, D] key order
        rscv = np.ascontiguousarray(
            ((xperm.astype(np.float64) @ w2) * scale).astype(np.float32))
        in_maps.append({
            "xT": _bf16(xperm.T), "xS": _bf16(xperm),
            "mT": M, "pT": Pm, "rsc": rscv, "bop": bo_p,
        })
    return in_maps


def assemble(results):
    out = np.empty((B, S, D), np.float32)
    for c in range(N_CORES):
        b, h = c // 2, c % 2
        out[b, h * SQ:(h + 1) * SQ] = results[c]["out"]
    return out


def kernel(x, Wq, bq, Wk, bk, Wv, bv, Wo, bo, **kwargs):
    nc = _get_nc()
    in_maps = make_in_maps(x, Wq, bq, Wk, bk, Wv, bv, Wo, bo)
    res = bass_utils.run_bass_kernel_spmd(nc, in_maps, core_ids=list(range(N_CORES)))
    return assemble(res.results)
